# revision 24
# baseline (speedup 1.0000x reference)
"""Trainium2 Bass kernel for nn_LorentzGraphConvolution (v7).

Row-sharded across 8 NeuronCores: core c owns rows [c*1536, (c+1)*1536) of
the attention matrix / output. Every core redundantly computes the tiny
linear phase (h, k for all N; q for its local rows) from broadcast inputs.

v7: the linear phase is EMISSION-INTERLEAVED with phase C so the
attention pipeline starts as soon as the first k tiles are transposed
(engine queues issue in order, so interleaving emission is what actually
overlaps the phases). Linear work runs in fused 24-tile batches
(matmuls -> square -> reduce -> copy -> per-batch normalization ->
per-batch transpose), with k-batch copies on DVE to keep ACT free for
sigmoids.

Phase C (per core): ACT sigmoid per j-pair [128,1024] from PSUM is the
wall (~157us); masks alternate between PE (additive BIG*adjT fp8
matmuls) and DVE (multiplicative bf16xfp8) pairs; MM1 row-packed pairs;
MM2 col-tiled pairs. adjT is host-pretransposed fp8 in octet groups
(4KB lines) on the GPSIMD SWDGE queue.

PSUM (shared, 8 banks): linear [128,1536] x1 (3) + attention pairs
[128,1024] x2 (4) + supT (1).
"""

import math
import os
import sys
from contextlib import ExitStack

for _p in ("/opt/trn_rl_repo", "/root/.axon_site/_ro/trn_rl_repo", "/root/.axon_site"):
    if os.path.isdir(_p) and _p not in sys.path:
        sys.path.insert(0, _p)

import ml_dtypes
import numpy as np

import concourse.bass as bass
import concourse.tile as tile
from concourse import bacc, bass_utils, masks, mybir
from concourse.tile import add_dep_helper

DT = mybir.dt
F32 = DT.float32
BF16 = DT.bfloat16
F8 = DT.float8e4
AF = mybir.ActivationFunctionType
ALU = mybir.AluOpType

N_FULL = 12288
D = 64
N_CORES = 8
R_FULL = N_FULL // N_CORES  # 1536 rows per core


def pick_big(sig_scale):
    """Smallest fp8_e4m3-exact value >= 45/sig_scale."""
    want = 45.0 / sig_scale
    v = float(np.float32(ml_dtypes.float8_e4m3(want)))
    while v < want:
        want *= 1.0625
        v = float(np.float32(ml_dtypes.float8_e4m3(want)))
    return v


def emit(tc, io, nn, rr, esc, esc_q, esc_k, sig_scale, sig_bias, big):
    nc = tc.nc
    TJ = nn // 128          # 96 global j tiles
    TL = rr // 128          # 12 local i tiles
    NCH = 3                 # i-chunks per core
    IC = rr // NCH          # 512 rows per chunk
    NOCT = TJ // 8          # 12 octets of j tiles per chunk
    NB = 24                 # linear batch size (tiles)
    assert IC == 512 and TJ % NB == 0

    ctx = ExitStack()

    const = ctx.enter_context(tc.tile_pool(name="const", bufs=1))
    persist = ctx.enter_context(tc.tile_pool(name="persist", bufs=1))
    slab = ctx.enter_context(tc.tile_pool(name="slab", bufs=1))
    flat = ctx.enter_context(tc.tile_pool(name="flat", bufs=2))
    kdp = ctx.enter_context(tc.tile_pool(name="kdp", bufs=1))
    oneshot = ctx.enter_context(tc.tile_pool(name="oneshot", bufs=1))
    wide = ctx.enter_context(tc.tile_pool(name="wide", bufs=2))
    small = ctx.enter_context(tc.tile_pool(name="small", bufs=4))
    oct_pool = ctx.enter_context(tc.tile_pool(name="octs", bufs=5))
    sig_pool = ctx.enter_context(tc.tile_pool(name="sig", bufs=3))
    out_pool = ctx.enter_context(tc.tile_pool(name="outp", bufs=2))
    psL = ctx.enter_context(tc.tile_pool(name="psL", bufs=1, space="PSUM"))
    psA = ctx.enter_context(tc.tile_pool(name="psA", bufs=2, space="PSUM"))
    psS = ctx.enter_context(tc.tile_pool(name="psS", bufs=1, space="PSUM"))

    # ---- constants / small inputs -------------------------------------
    xqT_s = const.tile([65, rr], BF16)
    nc.sync.dma_start(xqT_s[:], io["xqT"][:])
    wT_s = const.tile([65, 64], BF16)
    nc.sync.dma_start(wT_s[:], io["wT"][:])
    wqT_s = const.tile([65, 64], BF16)
    nc.sync.dma_start(wqT_s[:], io["wqT"][:])
    wkT_s = const.tile([65, 64], BF16)
    nc.sync.dma_start(wkT_s[:], io["wkT"][:])
    bigI = const.tile([128, 128], F8)
    last_in_dma = nc.sync.dma_start(bigI[:], io["bigi"][:])
    xT_s = flat.tile([65, nn], BF16, tag="flat")
    NXS = 4
    for xs in range(NXS):
        w0 = xs * (nn // NXS)
        nc.sync.dma_start(xT_s[:, w0:w0 + nn // NXS],
                          io["xT"][:, w0:w0 + nn // NXS])
    ident = const.tile([64, 64], F32)
    masks.make_identity(nc, ident[:])
    sig_bias_big = const.tile([128, 1], F32)
    nc.vector.memset(sig_bias_big[:], sig_bias - big * sig_scale)
    sig_bias_ap = const.tile([128, 1], F32)
    nc.vector.memset(sig_bias_ap[:], sig_bias)
    I32 = DT.int32
    magic = const.tile([128, 1], I32)
    nc.vector.memset(magic[:], 0x5F3759DF)

    def fast_rsqrt(dst, x, tmp_pool, nb, tag):
        """dst = 1/sqrt(x), bit-trick + 2 Newton iterations (DVE only)."""
        xi = x.bitcast(I32)
        sh = tmp_pool.tile([128, nb], I32, tag=tag + "sh", name="sh", bufs=2)
        nc.vector.tensor_scalar(sh[:], xi, 1, None, ALU.arith_shift_right)
        y = dst
        nc.vector.tensor_tensor(y.bitcast(I32), magic[:].to_broadcast((128, nb)),
                                sh[:], ALU.subtract)
        for _ in range(2):
            ysq = tmp_pool.tile([128, nb], F32, tag=tag + "ysq", name="ysq",
                                bufs=2)
            nc.vector.tensor_tensor(ysq[:], y, y, ALU.mult)
            t = tmp_pool.tile([128, nb], F32, tag=tag + "t", name="t", bufs=2)
            nc.vector.tensor_tensor(t[:], ysq[:], x, ALU.mult)
            w = tmp_pool.tile([128, nb], F32, tag=tag + "w", name="w", bufs=2)
            nc.vector.tensor_scalar(w[:], t[:], -0.5, 1.5, ALU.mult, ALU.add)
            yn = tmp_pool.tile([128, nb], F32, tag=tag + "yn", name="yn",
                               bufs=2)
            nc.vector.tensor_tensor(yn[:], y, w[:], ALU.mult)
            y = yn[:]
        nc.vector.tensor_copy(dst, y)

    hpad = persist.tile([128, TJ * 128], BF16)
    hpad3 = hpad.rearrange("p (t c) -> p t c", c=128)
    kT_stk = persist.tile([128, (TJ // 2) * 128], BF16)
    qmT_full = persist.tile([128, TL * 128], BF16)

    def lin_batch(lhsT_fn, rhs_w, esc_, neg, dest3, s0, nb, pref, ones_col,
                  copy_dve):
        """Fused linear batch: matmuls + Lorentz normalization + in-place
        finish for tiles [s0, s0+nb) of the destination slab."""
        ps = psL.tile([128, NB * 64], F32, tag="linps", name="linps")
        ps = ps[:, : nb * 64]
        ps3 = ps.rearrange("p (t d) -> p t d", d=64)
        for u in range(nb):
            nc.tensor.matmul(ps[:, u * 64:(u + 1) * 64], lhsT_fn(s0 + u),
                             rhs_w, start=True, stop=True)
        sqf = wide.tile([128, NB * 64], F32, tag="sqw", name="sqw")
        sqf = sqf[:, : nb * 64]
        nc.scalar.activation(sqf, ps, AF.Square)
        sqf3 = sqf.rearrange("p (t d) -> p t d", d=64)
        sq = slab.tile([128, nb], F32, tag=pref + "sq", name=pref + "sq")
        nc.vector.tensor_reduce(sq[:], sqf3[:, :, 1:64],
                                axis=mybir.AxisListType.X, op=ALU.add)
        logit = slab.tile([128, nb], F32, tag=pref + "lg", name=pref + "lg")
        nc.vector.tensor_copy(logit[:], ps3[:, :, 0])
        dsl = dest3[:, s0:s0 + nb, 0:64]
        if copy_dve:
            nc.vector.tensor_copy(dsl, ps3)
        else:
            nc.scalar.activation(dsl, ps3, AF.Copy)
        # normalization math on [128, nb]
        sg = slab.tile([128, nb], F32, tag=pref + "sg", name=pref + "sg")
        nc.scalar.activation(sg[:], logit[:], AF.Sigmoid)
        time = slab.tile([128, nb], F32, tag=pref + "tm", name=pref + "tm")
        a, c0 = (-esc_, -1.1) if neg else (esc_, 1.1)
        nc.vector.tensor_scalar(time[:], sg[:], a, c0, ALU.mult, ALU.add)
        sqc = slab.tile([128, nb], F32, tag=pref + "sc", name=pref + "sc")
        nc.vector.tensor_scalar_max(sqc[:], sq[:], 1e-8)
        t2 = slab.tile([128, nb], F32, tag=pref + "t2", name=pref + "t2")
        nc.vector.tensor_tensor(t2[:], time[:], time[:], ALU.mult)
        rec = slab.tile([128, nb], F32, tag=pref + "rc", name=pref + "rc")
        nc.vector.reciprocal(rec[:], sqc[:])
        ratio = slab.tile([128, nb], F32, tag=pref + "ra", name=pref + "ra")
        nc.vector.scalar_tensor_tensor(ratio[:], t2[:], -1.0, rec[:],
                                       ALU.add, ALU.mult)
        rsq = slab.tile([128, nb], F32, tag=pref + "rq", name=pref + "rq")
        fast_rsqrt(rsq[:], ratio[:], slab, nb, pref + "fq")
        sqs = slab.tile([128, nb], F32, tag=pref + "ss", name=pref + "ss")
        nc.vector.tensor_tensor(sqs[:], ratio[:], rsq[:], ALU.mult)
        # in-place finish
        nc.vector.tensor_tensor(dsl, dsl,
                                sqs[:].to_broadcast((128, nb, 64)), ALU.mult)
        nc.vector.tensor_copy(dest3[:, s0:s0 + nb, 0], time[:])
        if ones_col:
            nc.vector.memset(dest3[:, s0:s0 + nb, 64], 1.0)

    # =========== phase C chunk machinery ==============================
    adjt2 = io["adjt"]

    class Chunk:
        def __init__(self, c):
            self.c = c
            self.supT = psS.tile([128, 512], F32, tag="supT", name="supT")
            self.qch = qmT_full[:, c * IC:(c + 1) * IC]
            self.pending = None
            self.prev_lo = self.prev_hi = None

        def _mm2(self, stop):
            rhs, jl = self.pending
            start = jl == 0
            sA = nc.tensor.matmul(self.supT[0:64, :], hpad3[:, jl, 0:64],
                                  rhs[:, 0:512], start=start, stop=stop,
                                  tile_position=(0, 0))
            if self.prev_lo is not None:
                add_dep_helper(sA.ins, self.prev_lo.ins, sync=False,
                               reason="supT lo order")
            self.prev_lo = sA
            sB = nc.tensor.matmul(self.supT[64:128, :],
                                  hpad3[:, jl + 1, 0:64], rhs[:, 512:1024],
                                  start=start, stop=stop,
                                  tile_position=(0, 64))
            if self.prev_hi is not None:
                add_dep_helper(sB.ins, self.prev_hi.ins, sync=False,
                               reason="supT hi order")
            self.prev_hi = sB

        def octets(self, o0, o1):
            c = self.c
            for o in range(o0, o1):
                oct = oct_pool.tile([128, 8 * 512], F8, tag="oct", name="oct")
                oct3 = oct.rearrange("p (t q) -> p t q", q=512)
                r0 = (c * NOCT + o) * 128
                odma = nc.gpsimd.dma_start(oct[:], adjt2[r0:r0 + 128, :])
                if c == 0 and o == 0:
                    add_dep_helper(odma.ins, last_in_dma.ins, sync=True,
                                   reason="inputs before adj prefetch")
                for pr in range(4):
                    jl = o * 8 + pr * 2
                    tp = jl // 2
                    pe_mask = pr % 2 == 0
                    attT = psA.tile([128, 1024], F32, tag="attT", name="attT")
                    mmA = nc.tensor.matmul(
                        attT[:, 0:512], kT_stk[0:64, tp * 128:(tp + 1) * 128],
                        self.qch[0:64, :], start=True, stop=not pe_mask,
                        tile_position=(0, 0))
                    mmB = nc.tensor.matmul(
                        attT[:, 512:1024],
                        kT_stk[64:128, tp * 128:(tp + 1) * 128],
                        self.qch[64:128, :], start=True, stop=not pe_mask,
                        tile_position=(64, 0))
                    if pe_mask:
                        mA = nc.tensor.matmul(attT[:, 0:512], bigI[:],
                                              oct3[:, 2 * pr, :], start=False,
                                              stop=True)
                        add_dep_helper(mA.ins, mmA.ins, sync=False,
                                       reason="mask after ip A")
                        mB = nc.tensor.matmul(attT[:, 512:1024], bigI[:],
                                              oct3[:, 2 * pr + 1, :],
                                              start=False, stop=True)
                        add_dep_helper(mB.ins, mmB.ins, sync=False,
                                       reason="mask after ip B")
                    sig_t = sig_pool.tile([128, 1024], BF16, tag="sig",
                                          name="sig_t")
                    nc.scalar.activation(
                        sig_t[:], attT[:], AF.Sigmoid,
                        bias=sig_bias_big[:] if pe_mask else sig_bias_ap[:],
                        scale=sig_scale)
                    if pe_mask:
                        rhs = sig_t[:]
                    else:
                        sm = sig_pool.tile([128, 1024], BF16, tag="sm",
                                           name="sm")
                        nc.vector.tensor_tensor(
                            sm[:], sig_t[:],
                            oct[:, (2 * pr) * 512:(2 * pr + 2) * 512],
                            ALU.mult)
                        rhs = sm[:]
                    if self.pending is not None:
                        self._mm2(stop=False)
                    self.pending = (rhs, jl)

        def final(self):
            self._mm2(stop=True)
            supT = self.supT
            lo_s = small.tile([64, 512], F32, tag="los", name="lo_s")
            nc.vector.tensor_copy(lo_s[:], supT[0:64, :])
            sup_s = small.tile([64, 512], F32, tag="sups", name="sup_s")
            nc.vector.tensor_tensor(sup_s[:], supT[64:128, :], lo_s[:],
                                    ALU.add)
            sq_all = out_pool.tile([128, 4 * 64], F32, tag="sqall",
                                   name="sq_all")
            sq_all3 = sq_all.rearrange("p (s d) -> p s d", d=64)
            o_raw = out_pool.tile([128, 4 * 64], F32, tag="oraw",
                                  name="o_raw")
            o_raw3 = o_raw.rearrange("p (s d) -> p s d", d=64)
            for s in range(4):
                supn = psL.tile([128, NB * 64], F32, tag="linps", name="supn")
                supn = supn[:, 0:64]
                nc.tensor.transpose(supn, sup_s[:, s * 128:(s + 1) * 128],
                                    ident[:])
                nc.scalar.activation(sq_all3[:, s, :], supn, AF.Square)
                nc.vector.tensor_copy(o_raw3[:, s, :], supn)
            tot4 = small.tile([128, 4], F32, tag="ftot", name="tot4")
            nc.vector.tensor_reduce(tot4[:], sq_all3,
                                    axis=mybir.AxisListType.X, op=ALU.add)
            inner4 = small.tile([128, 4], F32, tag="finn", name="inner4")
            nc.vector.scalar_tensor_tensor(inner4[:], sq_all3[:, :, 0], -2.0,
                                           tot4[:], ALU.mult, ALU.add)
            negv = small.tile([128, 4], F32, tag="fneg", name="negv")
            nc.vector.tensor_scalar_mul(negv[:], inner4[:], -1.0)
            absv = small.tile([128, 4], F32, tag="fabs", name="absv")
            nc.vector.tensor_tensor(absv[:], inner4[:], negv[:], ALU.max)
            clip4 = small.tile([128, 4], F32, tag="fclip", name="clip4")
            nc.vector.tensor_scalar_max(clip4[:], absv[:], 1e-8)
            rs4 = small.tile([128, 4], F32, tag="frs", name="rs4")
            fast_rsqrt(rs4[:], clip4[:], small, 4, "ff")
            o_t = out_pool.tile([128, 4 * 64], F32, tag="otile", name="o_t")
            o_t3 = o_t.rearrange("p (s d) -> p s d", d=64)
            nc.vector.tensor_tensor(o_t3[:], o_raw3[:],
                                    rs4[:].to_broadcast((128, 4, 64)),
                                    ALU.mult)
            c = self.c
            nc.sync.dma_start(
                io["out"][c * IC:(c + 1) * IC, :].rearrange(
                    "(s p) d -> p s d", p=128), o_t3[:])

    # =========== interleaved emission schedule ========================
    hT_flat = flat.tile([128, TJ * 128], BF16, tag="flat")
    hT3 = hT_flat.rearrange("p (t n) -> p t n", n=128)
    kdense = kdp.tile([128, TJ * 64], BF16, tag="kd")
    kdense3 = kdense.rearrange("p (t d) -> p t d", d=64)
    kT3 = kT_stk.rearrange("p (t n) -> p t n", n=128)

    # hq (local) + its transpose -- fully independent of h
    hqpad = oneshot.tile([128, TL * 128], BF16, tag="hq")
    hqpad3 = hqpad.rearrange("p (t c) -> p t c", c=128)
    lin_batch(lambda t: xqT_s[:, t * 128:(t + 1) * 128], wT_s[:], esc, False,
              hqpad3, 0, TL, "hq", True, False)
    hqT_flat = oneshot.tile([128, TL * 128], BF16, tag="hqT")
    nc.sync.dma_start(hqT_flat.rearrange("p (t n) -> p t n", n=128),
                      hqpad[:], transpose=True)

    # h batch 0 + transpose
    lin_batch(lambda t: xT_s[:, t * 128:(t + 1) * 128], wT_s[:], esc, False,
              hpad3, 0, NB, "h0", True, False)
    nc.sync.dma_start(hT3[:, 0:NB, :], hpad[:, 0:NB * 128], transpose=True)

    # qm + its transpose
    qm_pad = oneshot.tile([128, TL * 128], BF16, tag="qmpad")
    qm_pad3 = qm_pad.rearrange("p (t c) -> p t c", c=128)
    lin_batch(lambda t: hqT_flat[0:65, t * 128:(t + 1) * 128], wqT_s[:],
              esc_q, True, qm_pad3, 0, TL, "qm", False, False)
    nc.vector.tensor_copy(qm_pad3[:, :, 64:128], qm_pad3[:, :, 0:64])
    nc.sync.dma_start(qmT_full.rearrange("p (t n) -> p t n", n=128),
                      qm_pad[:], transpose=True)

    # k batch 0 + transpose
    lin_batch(lambda t: hT_flat[0:65, t * 128:(t + 1) * 128], wkT_s[:],
              esc_k, False, kdense3, 0, NB, "k0", False, True)
    nc.sync.dma_start(kT3[:, 0:NB // 2, :], kdense[:, 0:NB * 64],
                      transpose=True)

    ck0 = Chunk(0)
    ck0.octets(0, 3)
    for b in range(1, TJ // NB):
        s0 = b * NB
        lin_batch(lambda t: xT_s[:, t * 128:(t + 1) * 128], wT_s[:], esc,
                  False, hpad3, s0, NB, "h%d" % b, True, False)
        nc.sync.dma_start(hT3[:, s0:s0 + NB, :],
                          hpad[:, s0 * 128:(s0 + NB) * 128], transpose=True)
        lin_batch(lambda t: hT_flat[0:65, t * 128:(t + 1) * 128], wkT_s[:],
                  esc_k, False, kdense3, s0, NB, "k%d" % b, False, True)
        nc.sync.dma_start(kT3[:, b * NB // 2:(b + 1) * NB // 2, :],
                          kdense[:, s0 * 64:(s0 + NB) * 64], transpose=True)
        ck0.octets(3 * b, 3 * (b + 1))
    ck0.final()

    for c in range(1, NCH):
        ck = Chunk(c)
        ck.octets(0, NOCT)
        ck.final()

    ctx.close()


def build(nn, rr, esc, esc_q, esc_k, sig_scale, sig_bias, num_devices=N_CORES):
    big = pick_big(sig_scale)
    nc = bacc.Bacc("TRN2", target_bir_lowering=False, debug=False,
                   num_devices=num_devices)
    nch = 3
    noct = nn // 128 // 8
    io = {
        "adjt": nc.dram_tensor("adjt", [nch * noct * 128, 8 * 512], F8,
                               kind="ExternalInput").ap(),
        "xT": nc.dram_tensor("xT", [65, nn], BF16, kind="ExternalInput").ap(),
        "xqT": nc.dram_tensor("xqT", [65, rr], BF16,
                              kind="ExternalInput").ap(),
        "wT": nc.dram_tensor("wT", [65, 64], BF16, kind="ExternalInput").ap(),
        "wqT": nc.dram_tensor("wqT", [65, 64], BF16,
                              kind="ExternalInput").ap(),
        "wkT": nc.dram_tensor("wkT", [65, 64], BF16,
                              kind="ExternalInput").ap(),
        "bigi": nc.dram_tensor("bigi", [128, 128], F8,
                               kind="ExternalInput").ap(),
        "out": nc.dram_tensor("out", [rr, 64], F32, kind="ExternalOutput").ap(),
    }
    with tile.TileContext(nc) as tc:
        emit(tc, io, nn, rr, esc, esc_q, esc_k, sig_scale, sig_bias, big)
    nc.compile()
    return nc


def make_in_maps(inputs, nn, rr, n_cores):
    bf = ml_dtypes.bfloat16
    f8 = ml_dtypes.float8_e4m3
    x = np.asarray(inputs["x"], np.float32)
    adj = np.ascontiguousarray(np.asarray(inputs["adj"], np.float32))
    W = np.asarray(inputs["W"], np.float32)
    b = np.asarray(inputs["b"], np.float32)
    Wq = np.asarray(inputs["Wq"], np.float32)
    bq = np.asarray(inputs["bq"], np.float32)
    Wk = np.asarray(inputs["Wk"], np.float32)
    bk = np.asarray(inputs["bk"], np.float32)

    att_scale = float(np.asarray(inputs["att_scale"], np.float32))
    big = pick_big(2.0 / att_scale)

    xT_ext = np.concatenate([x.T, np.ones((1, nn), np.float32)], 0).astype(bf)
    wT_ext = np.concatenate([W.T, b[None, :]], 0).astype(bf)
    wqT_ext = np.concatenate([Wq.T, bq[None, :]], 0).astype(bf)
    wkT_ext = np.concatenate([Wk.T, bk[None, :]], 0).astype(bf)
    bigI = (np.eye(128, dtype=np.float32) * big).astype(f8)

    in_maps = []
    for c in range(n_cores):
        r0 = c * rr
        slab = adj[r0:r0 + rr]                       # [1536, 12288]
        # adjt[(ch*12+o)*128+p, t*512+q] = slab[ch*512+q, (o*8+t)*128+p]
        a6 = slab.reshape(3, 512, 12, 8, 128).transpose(0, 2, 4, 3, 1)
        adjt = np.ascontiguousarray(a6.reshape(3 * 12 * 128, 8 * 512)).astype(f8)
        in_maps.append({
            "adjt": adjt,
            "xT": np.ascontiguousarray(xT_ext),
            "xqT": np.ascontiguousarray(xT_ext[:, r0:r0 + rr]),
            "wT": wT_ext,
            "wqT": wqT_ext,
            "wkT": wkT_ext,
            "bigi": bigI,
        })
    return in_maps


def consts_from_inputs(inputs):
    scale = float(np.asarray(inputs["scale"], np.float32))
    scale_q = float(np.asarray(inputs["scale_q"], np.float32))
    scale_k = float(np.asarray(inputs["scale_k"], np.float32))
    att_bias = float(np.asarray(inputs["att_bias"], np.float32))
    att_scale = float(np.asarray(inputs["att_scale"], np.float32))
    esc = math.exp(scale)
    esc_q = math.exp(scale_q)
    esc_k = math.exp(scale_k)
    sig_scale = 2.0 / att_scale
    sig_bias = 2.0 / att_scale + att_bias
    return esc, esc_q, esc_k, sig_scale, sig_bias


def kernel(**inputs):
    nn, rr = N_FULL, R_FULL
    consts = consts_from_inputs(inputs)
    nc = build(nn, rr, *consts)
    in_maps = make_in_maps(inputs, nn, rr, N_CORES)
    res = bass_utils.run_bass_kernel_spmd(nc, in_maps,
                                          core_ids=list(range(N_CORES)))
    return np.concatenate([res.results[c]["out"] for c in range(N_CORES)],
                          axis=0)
